# revision 18
# baseline (speedup 1.0000x reference)
"""Trainium2 Bass kernel for nn_PrototypicalGeometricLoss.

Strategy (8 NeuronCores, single NEFF launch):
  - Data-parallel streaming: each core mean-pools + L2-normalizes its B/8 = 512
    batch rows of geometric_stream (the memory-bound 134 MB read).  gs is
    viewed as [2048, 2048] quarter-rows; the stream is cast to bf16 in-flight
    (SWDGE DMA) and pooling over s runs on the PE as 16 accumulating bf16
    matmuls per 32-batch tile against a constant selection matrix.  DVE/GPSIMD
    stay idle during the stream.
  - Pooled/normalized embeddings are transposed on-chip ([D, b] bf16 tiles)
    and AllGathered in 4 pipelined chunks (one per local b-tile) so collective
    latency hides under the stream; every core ends with gT = [128, 4096] bf16.
  - Key split: classes whose prototype is NOT updated this step (no members in
    the batch) keep their exact old prototype, so their distance contributions
    are computed DURING the stream, per AllGather chunk, against the
    host-supplied (per-shard-reordered, updated-classes-first) prototypesT.
    Only the ~NUPAD updated classes per shard need the post-collective pass.
  - Prototype EMA update is class-sharded and covers only the updated block:
    gather/scan/gather segment sums (member columns of gT, scaled by the
    per-member EMA factor via a PE-broadcast row, prefix-scan, endpoint
    gather, shifted subtract), then ptil = 0.9*pT_old + sums, column norms via
    Square + ones-matmul, reciprocal row broadcast by PE outer product, one
    DVE multiply -> pT [D, NUPAD] bf16.  No transposes.
  - Post-collective distances: all 4096 b x NUPAD updated classes per core,
    batched 3 b-tiles per PSUM tile, one ACT Sqrt+accum per batch
    (d = sqrt(2 - 2*g.p); g pre-scaled by (1-1e-6) keeps the argument > 0).
  - Own-class dot products reuse the member gather for the g side, gather the
    p side from pT, reduce via PE matmul diagonals; raw dots return to the
    host which sqrts them in float64 for exact intra_loss.
  - Host combines per-core partial sums (float64) into the six loss scalars.
"""

import functools
import sys

sys.path.insert(0, "/opt/trn_rl_repo")

import numpy as np
import ml_dtypes

import concourse.bass as bass  # noqa: F401
import concourse.bacc as bacc
import concourse.mybir as mybir
from concourse import tile
from concourse.bass_utils import run_bass_kernel_spmd

N_CORES = 8
B, S, D, C = 4096, 64, 128, 10000
BSH = B // N_CORES           # 512 batch rows per core
LTB = BSH // 128             # 4 local b-tiles
NTB = B // 128               # 32 global b-tiles
CSH = C // N_CORES           # 1250 classes per core
CPAD = 1280                  # padded to 10 x 128
GCOL = NTB * 128             # 4096 columns of gT (last col is zero)
MOM = 0.9
GSCALE = 1.0 - 1e-6          # keeps 2 - 2*g.p strictly positive
EPS = 1e-12
QR = 2048                    # gs quarter-row length (16 s x 128 d)

f32 = mybir.dt.float32
f32r = mybir.dt.float32r
bf16 = mybir.dt.bfloat16
i16 = mybir.dt.int16
AF = mybir.ActivationFunctionType
ALU = mybir.AluOpType
AX = mybir.AxisListType


def _wrap16(flat):
    """Lay a flat index list out in the GPSIMD wrapped-by-16 format."""
    n = flat.shape[0]
    assert n % 16 == 0
    w = flat.reshape(n // 16, 16).T.astype(np.int16)   # [16, n//16]
    return np.tile(w, (8, 1))                          # [128, n//16]


@functools.lru_cache(maxsize=16)
def _build(NOCH, NUPAD, NBK, NBREAL, use_f32r=True, upto=99, unroll=1,
           skip_ag=False):
    """Build + compile the SPMD program.

    NOCH = own-dot chunks of 128; NUPAD = padded updated-classes-per-shard
    block; NBK/NBREAL = padded/real count of globally-never-updated classes
    (the b-sharded bulk pass)."""
    NOWN = NOCH * 128
    MGL = NOWN + 16          # member-gather length (leading zero + pads)
    EG2 = NUPAD + 16         # endpoint-gather length
    UPB = max(1, 1536 // NUPAD)         # upd b-tiles batched per PSUM tile
    # bulk (local b-tile x never-updated classes) PSUM chunks of 1536 cols
    bulk_chunks = []                     # (c0, width) into prTg
    for c0 in range(0, NBK, 1536):
        bulk_chunks.append((c0, min(1536, NBK - c0)))
    upd_groups = [list(range(b0, min(b0 + UPB, NTB)))
                  for b0 in range(0, NTB, UPB)]
    NBG = LTB * len(bulk_chunks)
    NAC = NBG + len(upd_groups)

    nc = bacc.Bacc("TRN2", target_bir_lowering=False, debug=False,
                   num_devices=N_CORES)

    gs = nc.dram_tensor("gs", [BSH * 4, QR], f32, kind="ExternalInput")
    prTb = nc.dram_tensor("prTb", [128, NUPAD], bf16, kind="ExternalInput")
    prTg = nc.dram_tensor("prTg", [128, NBK], bf16, kind="ExternalInput")
    sv = nc.dram_tensor("sv", [128, BSH // 128], f32, kind="ExternalInput")
    ssm = nc.dram_tensor("ssm", [1, MGL], f32, kind="ExternalInput")
    mgi = nc.dram_tensor("mgi", [128, MGL // 16], i16, kind="ExternalInput")
    e2i = nc.dram_tensor("e2i", [128, EG2 // 16], i16, kind="ExternalInput")
    opi = nc.dram_tensor("opi", [128, NOWN // 16], i16, kind="ExternalInput")
    idn = nc.dram_tensor("idn", [128, 128], f32, kind="ExternalInput")
    osel = nc.dram_tensor("osel", [128, 32], f32, kind="ExternalInput")
    po = nc.dram_tensor("po", [128, 4], f32, kind="ExternalOutput")
    oo = nc.dram_tensor("oo", [128, NOCH], f32, kind="ExternalOutput")

    slab_bufs = 8 if NOCH <= 10 else 4
    with tile.TileContext(nc) as tc:
        with (
            tc.tile_pool(name="dram", bufs=1, space="DRAM") as dram,
            tc.tile_pool(name="consts", bufs=1) as consts,
            tc.tile_pool(name="gbig", bufs=1) as gbig,
            tc.tile_pool(name="slab", bufs=slab_bufs) as slabp,
            tc.tile_pool(name="norm", bufs=2) as normp,
            tc.tile_pool(name="glocp", bufs=4) as glocp,
            tc.tile_pool(name="ps_small", bufs=2, space="PSUM") as ps_small,
            tc.tile_pool(name="ps_big", bufs=2, space="PSUM") as ps_big,
            tc.tile_pool(name="dscr", bufs=2) as dscrp,
            tc.tile_pool(name="outs", bufs=1) as outsp,
        ):
            for it in range(unroll):
                if it > 0:
                    tc.strict_bb_all_engine_barrier()
                ag_ins = [dram.tile([128, 128], bf16, name=f"ag_in{t}",
                                    tag=f"ag_in{t}") for t in range(LTB)]
                ag_outs = [dram.tile([N_CORES, 128, 128], bf16,
                                     name=f"ag_out{t}", addr_space="Shared",
                                     tag=f"ag_out{t}") for t in range(LTB)]
                ident = consts.tile([128, 128], f32, name="ident")
                nc.sync.dma_start(ident[:, :], idn[:, :])
                osel_sb = consts.tile([128, 32], bf16, name="osel_sb")
                nc.gpsimd.dma_start(osel_sb[:, :], osel[:, :])
                mgi_sb = consts.tile([128, MGL // 16], i16, name="mgi_sb")
                nc.sync.dma_start(mgi_sb[:, :], mgi[:, :])
                e2i_sb = consts.tile([128, EG2 // 16], i16, name="e2i_sb")
                nc.sync.dma_start(e2i_sb[:, :], e2i[:, :])
                opi_sb = consts.tile([128, NOWN // 16], i16, name="opi_sb")
                nc.sync.dma_start(opi_sb[:, :], opi[:, :])
                sv_sb = consts.tile([128, BSH // 128], f32, name="sv_sb")
                nc.sync.dma_start(sv_sb[:, :], sv[:, :])
                ssm_sb = consts.tile([1, MGL], f32, name="ssm_sb")
                nc.sync.dma_start(ssm_sb[:, :], ssm[0:1, :])
                bias2 = consts.tile([128, 1], f32, name="bias2")
                nc.vector.memset(bias2[:, :], 2.0)
                onescol = consts.tile([128, 1], f32, name="onescol")
                nc.vector.memset(onescol[:, :], 1.0)
                onesrow = consts.tile([1, 128], f32, name="onesrow")
                nc.vector.memset(onesrow[:, :], 1.0)
                prT_sb = consts.tile([128, NUPAD], bf16, name="prT_sb")
                nc.sync.dma_start(prT_sb[:, :], prTb[:, :])
                prTg_sb = consts.tile([128, NBK], bf16, name="prTg_sb")
                nc.sync.dma_start(prTg_sb[:, :], prTg[:, :])

                gT = gbig.tile([128, GCOL + 1], bf16, name="gT")
                nc.vector.memset(gT[:, GCOL:GCOL + 1], 0.0)
                gTw = gbig.tile([128, GCOL + 1], f32, name="gTw")
                nc.vector.memset(gTw[:, GCOL:GCOL + 1], 0.0)
                acc = outsp.tile([128, NAC], f32, name="acc")
                if upto < 1:
                    nc.vector.memset(acc[:, 0:NBG], 0.0)

                # ---- Phase A: stream + PE-pool + normalize + transpose
                glocs = []
                for t in range(LTB if upto >= 1 else 0):
                    pooled = ps_small.tile([128, 128], f32, name="pooled",
                                           tag="pst")
                    for tl in range(4):
                        sl = slabp.tile([128, QR], bf16, name="sl", tag="slab")
                        r0 = t * 512 + tl * 128
                        nc.gpsimd.dma_start(sl[:, :], gs[r0:r0 + 128, :])
                        for q in range(16):
                            nc.tensor.matmul(
                                pooled[tl * 32:(tl + 1) * 32, :],
                                osel_sb[:, :],
                                sl[:, q * 128:(q + 1) * 128],
                                start=(q == 0), stop=(q == 15),
                                tile_position=(0, tl * 32))
                    scr = normp.tile([128, 128], f32, name="scr", tag="scr")
                    ssq = normp.tile([128, 1], f32, name="ssq", tag="ssq")
                    nc.scalar.activation(scr[:, :], pooled[:, :], AF.Square,
                                         accum_out=ssq[:, :])
                    nrm = normp.tile([128, 1], f32, name="nrm", tag="nrm")
                    nc.scalar.activation(nrm[:, :], ssq[:, :], AF.Sqrt)
                    nc.vector.tensor_scalar_max(nrm[:, :], nrm[:, :], EPS)
                    rcp = normp.tile([128, 1], f32, name="rcp", tag="rcp")
                    nc.vector.reciprocal(rcp[:, :], nrm[:, :])
                    gn = normp.tile([128, 128], f32, name="gn", tag="gn")
                    nc.vector.tensor_scalar(gn[:, :], pooled[:, :], rcp[:, :],
                                            GSCALE, ALU.mult, ALU.mult)
                    pst = ps_small.tile([128, 128], f32, name="pst", tag="pst")
                    nc.tensor.transpose(pst[:, :], gn[:, :], ident[:, :])
                    gloc = glocp.tile([128, 128], bf16, name="gloc",
                                      tag="gloc")
                    nc.scalar.activation(gloc[:, :], pst[:, :], AF.Copy)
                    nc.sync.dma_start(ag_ins[t][:, :], gloc[:, :])
                    glocs.append(gloc)

                # ---- bulk: local b-tiles x globally-never-updated classes
                for t, gloc in enumerate(glocs):
                    for ci, (c0, cw) in enumerate(bulk_chunks):
                        psb = ps_big.tile([128, 1536], f32, name="psb",
                                          tag="psb")
                        for d0 in range(0, cw, 512):
                            dn = min(512, cw - d0)
                            nc.tensor.matmul(
                                psb[:, d0:d0 + dn],
                                gloc[:, :],
                                prTg_sb[:, c0 + d0:c0 + d0 + dn],
                                start=True, stop=True)
                        rw = min(cw, max(0, NBREAL - c0))
                        dscb = dscrp.tile([128, 1536], bf16, name="dscb",
                                          tag="dsc")
                        gidx = t * len(bulk_chunks) + ci
                        nc.scalar.activation(dscb[:, 0:rw], psb[:, 0:rw],
                                             AF.Sqrt, bias=bias2[:, :],
                                             scale=-2.0,
                                             accum_out=acc[:, gidx:gidx + 1])

                # ---- simplex volume partials
                out_sb = outsp.tile([128, 4], f32, name="out_sb")
                nc.vector.tensor_reduce(out_sb[:, 1:2], sv_sb[:, :], AX.X, ALU.add)
                junk1 = outsp.tile([128, BSH // 128], f32, name="junk1")
                nc.vector.scalar_tensor_tensor(junk1[:, :], sv_sb[:, :], 1.0,
                                               sv_sb[:, :], ALU.mult, ALU.mult,
                                               accum_out=out_sb[:, 2:3])
                nc.vector.memset(out_sb[:, 3:4], 0.0)

                # ---- Phase B: pipelined per-b-tile AllGathers; assemble gT
                if upto < 1:
                    for t in range(LTB):
                        nc.sync.dma_start(ag_ins[t][:, :], gT[:, 0:128])
                if upto < 2:
                    nc.vector.memset(gT[:, 0:GCOL], 0.0)
                    nc.vector.memset(gTw[:, 0:GCOL], 0.0)
                gTv = gT[:, 0:GCOL].rearrange("p (j r) -> p j r", j=N_CORES)
                gTwv = gTw[:, 0:GCOL].rearrange("p (j r) -> p j r", j=N_CORES)
                for t in range(LTB if upto >= 2 else 0):
                    if not skip_ag:
                        nc.gpsimd.collective_compute(
                            "AllGather", ALU.bypass,
                            replica_groups=[list(range(N_CORES))],
                            ins=[ag_ins[t].opt()], outs=[ag_outs[t].opt()])
                    nc.sync.dma_start(
                        gTv[:, :, t * 128:(t + 1) * 128],
                        ag_outs[t][:, :, :].rearrange("j p c -> p j c"))
                    nc.scalar.activation(
                        gTwv[:, :, t * 128:(t + 1) * 128],
                        gTv[:, :, t * 128:(t + 1) * 128], AF.Copy)

                # ---- Phase C: segment sums via gather + scale + scan + gather
                mems = gbig.tile([128, MGL], f32, name="mems")
                cum = gbig.tile([128, MGL], f32, name="cum")
                fx = gbig.tile([128, EG2], f32, name="fx")
                suma = gbig.tile([128, NUPAD], f32, name="suma")
                if upto >= 3:
                    nc.gpsimd.ap_gather(mems[:, :], gTw[:, :], mgi_sb[:, :],
                                        channels=128, num_elems=GCOL + 1, d=1,
                                        num_idxs=MGL)
                    memss = gbig.tile([128, MGL], f32, name="memss")
                    for b0 in range(0, MGL, 1024):
                        bn = min(1024, MGL - b0)
                        ssb = ps_big.tile([128, 1024], f32, name="ssb",
                                          tag="psb")
                        for c0 in range(0, bn, 512):
                            cn = min(512, bn - c0)
                            nc.tensor.matmul(
                                ssb[:, c0:c0 + cn],
                                onesrow[:, :],
                                ssm_sb[:, b0 + c0:b0 + c0 + cn],
                                start=True, stop=True)
                        nc.vector.tensor_tensor(memss[:, b0:b0 + bn],
                                                mems[:, b0:b0 + bn],
                                                ssb[:, 0:bn], ALU.mult)
                    nc.vector.tensor_tensor_scan(cum[:, :], memss[:, :],
                                                 memss[:, :], 0.0,
                                                 ALU.add, ALU.bypass)
                    nc.gpsimd.ap_gather(fx[:, 0:EG2], cum[:, :], e2i_sb[:, :],
                                        channels=128, num_elems=MGL, d=1,
                                        num_idxs=EG2)
                    nc.vector.tensor_sub(suma[:, 0:NUPAD], fx[:, 1:NUPAD + 1],
                                         fx[:, 0:NUPAD])
                else:
                    nc.vector.memset(mems[:, :], 0.0)
                    nc.vector.memset(suma[:, :], 0.0)

                # ---- Phase D: prototype EMA + renorm -> pT [D, NUPAD] bf16
                ptil = gbig.tile([128, NUPAD], f32, name="ptil")
                sqt = gbig.tile([128, NUPAD], f32, name="sqt")
                nrow = outsp.tile([1, NUPAD], f32, name="nrow")
                rrow = outsp.tile([1, NUPAD], f32, name="rrow")
                pT = gbig.tile([128, NUPAD], f32, name="pT")
                pTb = gbig.tile([128, NUPAD], bf16, name="pTb")
                if upto >= 4:
                    nc.vector.scalar_tensor_tensor(ptil[:, :],
                                                   prT_sb[:, 0:NUPAD], MOM,
                                                   suma[:, :],
                                                   ALU.mult, ALU.add)
                    nc.scalar.activation(sqt[:, :], ptil[:, :], AF.Square)
                    nps = ps_small.tile([128, NUPAD], f32, name="nps",
                                        tag="pst")
                    for c0 in range(0, NUPAD, 512):
                        cn = min(512, NUPAD - c0)
                        nc.tensor.matmul(nps[0:1, c0:c0 + cn],
                                         onescol[:, :],
                                         sqt[:, c0:c0 + cn],
                                         start=True, stop=True)
                    nc.scalar.activation(nrow[:, :], nps[0:1, :], AF.Sqrt)
                    nc.vector.tensor_scalar_max(nrow[:, :], nrow[:, :], EPS)
                    nc.vector.reciprocal(rrow[:, :], nrow[:, :])
                    rps = ps_small.tile([128, NUPAD], f32, name="rps",
                                        tag="pst")
                    for c0 in range(0, NUPAD, 512):
                        cn = min(512, NUPAD - c0)
                        nc.tensor.matmul(rps[:, c0:c0 + cn],
                                         onesrow[:, :],
                                         rrow[:, c0:c0 + cn],
                                         start=True, stop=True)
                    nc.vector.tensor_tensor(pT[:, :], ptil[:, :],
                                            rps[:, :], ALU.mult)
                    nc.scalar.activation(pTb[:, :], pT[:, :], AF.Copy)
                else:
                    nc.vector.memset(pT[:, :], 0.0)
                    nc.vector.memset(pTb[:, :], 0.0)

                # ---- Phase E: own-class raw dot products (g side = mems)
                opg = gbig.tile([128, NOWN], f32, name="opg")
                if upto >= 5:
                    nc.gpsimd.ap_gather(opg[:, :], pT[:, :], opi_sb[:, :],
                                        channels=128, num_elems=NUPAD, d=1,
                                        num_idxs=NOWN)
                dots = outsp.tile([128, NOCH], f32, name="dots")
                junk2 = outsp.tile([128, 128], f32, name="junk2")
                if upto < 5:
                    nc.vector.memset(dots[:, :], 0.0)
                    nc.vector.memset(opg[:, :], 0.0)
                for cc in range(NOCH if upto >= 5 else 0):
                    psd = ps_small.tile([128, 128], f32, name="psd", tag="pst")
                    nc.tensor.matmul(psd[:, :],
                                     mems[:, 1 + cc * 128:1 + (cc + 1) * 128],
                                     opg[:, cc * 128:(cc + 1) * 128],
                                     start=True, stop=True)
                    nc.vector.scalar_tensor_tensor(
                        junk2[:, :], psd[:, :], 1.0, ident[:, :],
                        ALU.mult, ALU.mult, accum_out=dots[:, cc:cc + 1])
                nc.sync.dma_start(oo[:, :], dots[:, :])

                # ---- Phase F: distances for the updated-class block
                if upto < 6:
                    nc.vector.memset(acc[:, NBG:NAC], 0.0)
                for ci, bts in enumerate(upd_groups if upto >= 6 else []):
                    nb = len(bts)
                    psu = ps_big.tile([128, UPB * NUPAD], f32, name="psu",
                                      tag="psb")
                    for k, bt in enumerate(bts):
                        nc.tensor.matmul(psu[:, k * NUPAD:(k + 1) * NUPAD],
                                         gT[:, bt * 128:(bt + 1) * 128],
                                         pTb[:, :],
                                         start=True, stop=True)
                    dscu = dscrp.tile([128, UPB * NUPAD], bf16, name="dscu",
                                      tag="dsc")
                    nc.scalar.activation(dscu[:, 0:nb * NUPAD],
                                         psu[:, 0:nb * NUPAD], AF.Sqrt,
                                         bias=bias2[:, :], scale=-2.0,
                                         accum_out=acc[:, NBG + ci:NBG + ci + 1])
                nc.vector.tensor_reduce(out_sb[:, 0:1], acc[:, :], AX.X, ALU.add)
                nc.sync.dma_start(po[:, :], out_sb[:, :])

    nc.compile()
    return nc


def _prep(geometric_stream, simplex_volumes, prototypes, labels):
    gs = np.ascontiguousarray(np.asarray(geometric_stream, dtype=np.float32))
    svol = np.ascontiguousarray(np.asarray(simplex_volumes, dtype=np.float32))
    pr = np.asarray(prototypes, dtype=np.float32)
    lab = np.asarray(labels).astype(np.int64).ravel()
    assert gs.shape == (B, S, D) and pr.shape == (C, D) and lab.shape == (B,)

    counts = np.bincount(lab, minlength=C)
    sscale = ((1.0 - MOM) / np.maximum(counts, 1.0)).astype(np.float32)

    shard_of = lab // CSH
    n_own = np.bincount(shard_of, minlength=N_CORES)
    NOCH = max(1, int(-(-n_own.max() // 128)))
    NOWN = NOCH * 128
    MGL = NOWN + 16

    # osel[p, m] = 1 iff p//4 == m  (sums the 4 s-quarters of batch m)
    osel = (np.arange(128)[:, None] // 4 == np.arange(32)[None, :])
    osel = osel.astype(np.float32)
    ident = np.eye(128, dtype=np.float32)

    shards = []
    n_u_max = 0
    for j in range(N_CORES):
        c0 = j * CSH
        bsel = np.nonzero(shard_of == j)[0]
        slots = lab[bsel] - c0
        upd = np.unique(slots)
        n_u_max = max(n_u_max, len(upd))
        order = np.concatenate([upd, np.setdiff1d(np.arange(CSH), upd)])
        inv = np.empty(CSH, dtype=np.int64)
        inv[order] = np.arange(CSH)
        nslots = inv[slots]
        srt = np.lexsort((bsel, nslots))
        shards.append((c0, bsel[srt], nslots[srt], order, len(upd)))
    NUPAD = max(128, int(-(-n_u_max // 128)) * 128)
    EG2 = NUPAD + 16

    # globally-never-updated classes: the b-sharded bulk block (replicated)
    never = np.nonzero(counts == 0)[0]
    NBREAL = len(never)
    NBK = max(128, int(-(-NBREAL // 128)) * 128)
    prg = np.zeros((NBK, D), dtype=np.float32)
    prg[:NBREAL] = pr[never]
    prTg = np.ascontiguousarray(prg.T).astype(ml_dtypes.bfloat16)

    in_maps = []
    own_b = []   # per core: batch indices in (class, b) order
    n_us = []    # per core: real updated-class count (pads add sqrt(2) each)
    for j in range(N_CORES):
        c0, bsel, nslots, order, n_u = shards[j]
        n_us.append(n_u)
        n_j = len(bsel)

        # member gather: [zero] + class-sorted member columns + zero pads
        mg = np.full(MGL, GCOL, dtype=np.int64)
        mg[1:1 + n_j] = bsel
        # per-member EMA scale (1-MOM)/count of its class; 0 for slot 0 + pads
        ssv = np.zeros(MGL, dtype=np.float32)
        ssv[1:1 + n_j] = sscale[lab[bsel]]
        # endpoint gather: position of cumulative sum after each class
        m_c = np.cumsum(np.bincount(nslots, minlength=NUPAD))
        e2 = np.zeros(EG2, dtype=np.int64)
        e2[1:NUPAD + 1] = m_c
        e2[NUPAD + 1:] = m_c[-1]
        # own-p gather: dense class slot per member (pads point at slot 0;
        # their dots are discarded by the host)
        opf = np.zeros(NOWN, dtype=np.int64)
        opf[:n_j] = nslots

        prj = np.zeros((NUPAD, D), dtype=np.float32)
        prj[:n_u] = pr[c0:c0 + CSH][order[:n_u]]
        prT = np.ascontiguousarray(prj.T).astype(ml_dtypes.bfloat16)

        in_maps.append({
            "gs": gs[BSH * j:BSH * (j + 1)].reshape(BSH * 4, QR),
            "prTb": prT,
            "prTg": prTg,
            "sv": svol[BSH * j:BSH * (j + 1)].reshape(128, BSH // 128),
            "ssm": ssv.reshape(1, MGL),
            "mgi": _wrap16(mg),
            "e2i": _wrap16(e2),
            "opi": _wrap16(opf),
            "idn": ident,
            "osel": osel,
        })
        own_b.append(bsel)

    return in_maps, own_b, NOCH, NUPAD, NBK, NBREAL, n_us


def _finish(results, own_b, NOCH, NUPAD, n_us):
    # pad slots in the updated block score sqrt(2 - 0) each; remove them
    pad_d = sum(NUPAD - n_u for n_u in n_us) * float(B) * np.sqrt(2.0)
    sum_d = -pad_d
    sum_v = 0.0
    sum_v2 = 0.0
    d_own_all = np.empty(B, dtype=np.float64)
    n_total = 0
    for j in range(N_CORES):
        po = results[j]["po"].astype(np.float64)
        oo = results[j]["oo"].astype(np.float64)
        sum_d += po[:, 0].sum()
        sum_v += po[:, 1].sum()
        sum_v2 += po[:, 2].sum()
        bsel = own_b[j]
        vals = oo.T.ravel()[:len(bsel)]          # chunk-major: i = c*128 + p
        gp = vals / GSCALE
        d_own_all[bsel] = np.sqrt(np.maximum(0.0, 2.0 - 2.0 * gp))
        n_total += len(bsel)
    assert n_total == B

    intra = d_own_all.mean()
    viol_all = 2.0 * B * C - sum_d
    viol_own = np.maximum(0.0, 2.0 - d_own_all).sum()
    inter = (viol_all - viol_own) / (B * (C - 1))
    mean_v = sum_v / B
    var_v = max((sum_v2 - B * mean_v * mean_v) / (B - 1), 0.0)
    vdl = -np.sqrt(var_v)
    cr = -mean_v
    total = 1.0 * intra + 2.0 * inter + 0.5 * vdl + 0.1 * cr
    return (np.float32(total), np.float32(intra), np.float32(inter),
            np.float32(vdl), np.float32(cr), np.float32(intra))


USE_F32R = True


def kernel(geometric_stream, simplex_volumes, prototypes, labels):
    in_maps, own_b, NOCH, NUPAD, NBK, NBREAL, n_us = _prep(
        geometric_stream, simplex_volumes, prototypes, labels)
    nc = _build(NOCH, NUPAD, NBK, NBREAL, USE_F32R)
    res = run_bass_kernel_spmd(nc, in_maps, core_ids=list(range(N_CORES)))
    return _finish(res.results, own_b, NOCH, NUPAD, n_us)
